# revision 18
# baseline (speedup 1.0000x reference)
"""Causal single-head attention (B=8, N=2048, D=H=1024, fp32) on 8 TRN2 cores.

Data-parallel: one batch element per NeuronCore. Fully SBUF-resident — no
DRAM spills. fp8-e4m3 with MatmulPerfMode.DoubleRow (two 128-deep contraction
slabs per matmul = 2x PE throughput) is used wherever max-abs accuracy
permits, tiered by how concentrated the causal attention can be:

  - queries 0..127   : bf16 everything (private head path) — they attend over
                       so few keys that fp8 score noise doesn't average out.
  - queries 128..511 : fp8-DR q/k projections + scores; bf16 probs/V.
  - queries 512..2047: fp8-DR q/k projections + scores, fp8 probs and fp8 V
                       (V rows >=512 also projected in fp8-DR).

Scores are computed transposed (S^T = K @ Q^T, [key, query]) so softmax
reduces over the partition (key) axis via 1-wide ones-matmuls, and the
normalization is folded into the output eviction as a per-partition scale.

ATTN_MODE=bf16 switches everything to bf16 (no fp8) as a fallback.
"""

import os
import sys
from contextlib import ExitStack

import numpy as np
import ml_dtypes

try:
    import concourse.bacc as bacc
except ImportError:  # pragma: no cover
    sys.path.insert(0, "/opt/trn_rl_repo")
    import concourse.bacc as bacc

import concourse.mybir as mybir
from concourse.tile import TileContext
from concourse.bass_utils import run_bass_kernel_spmd

# bass_utils imports antenv.axon_hooks when BASS_TRACE is set; provide a stub
# so tracing degrades gracefully instead of crashing if the module is absent.
try:
    import antenv.axon_hooks  # noqa: F401
except ImportError:  # pragma: no cover
    import types

    _m = types.ModuleType("antenv.axon_hooks")
    _m._hook = None
    _m.set_axon_ntff_profile_hook = lambda h: setattr(_m, "_hook", h)
    _m.get_axon_ntff_profile_hook = lambda: _m._hook
    sys.modules["antenv.axon_hooks"] = _m

B, N, D, H = 8, 2048, 1024, 1024
P = 128
DT = D // P          # 8 contraction tiles for the projections
DP = DT // 2         # 4 DoubleRow d-pairs
HT = H // P          # 8 h-tiles
HP = HT // 2         # 4 DoubleRow h-pairs
NT = N // P          # 16 sequence tiles of 128
NP2 = NT // 2        # 8 key-tile pairs
IT = N // 512        # 4 query tiles of 512
SCALE = 1.0 / np.sqrt(float(H))

F32 = mybir.dt.float32
BF16 = mybir.dt.bfloat16
FP8 = mybir.dt.float8e4

LAST_RESULT = None  # BassKernelResults of the most recent kernel() call
_CACHE = {}


def build_program(mode: str):
    fp8 = mode == "fp8"
    qk_dt = FP8 if fp8 else BF16
    DR = mybir.MatmulPerfMode.DoubleRow if fp8 else None

    nc = bacc.Bacc("TRN2", target_bir_lowering=False, debug=False)

    # Host packs x^T and Wq/Wk/Wv into DoubleRow d-pair layout:
    #   xT8[m, p, i, n] = x^T[(2m+i)*128 + p, n],  W8[m, p, i, h] likewise.
    xT8 = nc.dram_tensor("xT8", [DP, P, 2, N], qk_dt, kind="ExternalInput")
    Wq8 = nc.dram_tensor("Wq8", [DP, P, 2, H], qk_dt, kind="ExternalInput")
    Wk8 = nc.dram_tensor("Wk8", [DP, P, 2, H], qk_dt, kind="ExternalInput")
    Wv8 = nc.dram_tensor("Wv8", [DP, P, 2, H], qk_dt, kind="ExternalInput")
    xTb = nc.dram_tensor("xTb", [DT, P, 512], BF16, kind="ExternalInput")
    Wvb = nc.dram_tensor("Wvb", [DT, P, H], BF16, kind="ExternalInput")
    Wqb = nc.dram_tensor("Wqb", [DT, P, H], BF16, kind="ExternalInput")
    Wkb = nc.dram_tensor("Wkb", [DT, P, H], BF16, kind="ExternalInput")
    bqT = nc.dram_tensor("bqT", [P, HT], F32, kind="ExternalInput")
    bkT = nc.dram_tensor("bkT", [P, HT], F32, kind="ExternalInput")
    bvB = nc.dram_tensor("bvB", [P, H], F32, kind="ExternalInput")
    out = nc.dram_tensor("out", [N, H], F32, kind="ExternalOutput")

    Exp = mybir.ActivationFunctionType.Exp
    Identity = mybir.ActivationFunctionType.Identity
    Copy = mybir.ActivationFunctionType.Copy

    with TileContext(nc) as tc:
        with ExitStack() as top:
            const = top.enter_context(tc.tile_pool(name="const", bufs=1))
            res = top.enter_context(tc.tile_pool(name="res", bufs=1))
            ps_s = top.enter_context(tc.tile_pool(name="pss", bufs=3, space="PSUM"))
            ps_rs = top.enter_context(tc.tile_pool(name="psrs", bufs=1, space="PSUM"))

            ones = const.tile([P, 1], BF16, tag="ones")
            nc.vector.memset(ones[:], 1.0)
            ones8 = const.tile([P, 2, 1], qk_dt, tag="ones8")
            nc.vector.memset(ones8[:], 1.0)
            bq_sb = const.tile([P, HT], F32, tag="bq")
            bk_sb = const.tile([P, HT], F32, tag="bk")

            # Resident activations. qt8/kt8 hold the DoubleRow h-pair layout
            # ([:, i, :] = h-tile 2m+i); v8p the key-pair layout ([:, i, :] =
            # v rows (2m+i)*128..); vt holds bf16 v rows 0..511 for t=0.
            qt8 = [res.tile([P, 2, N], qk_dt, tag=f"qt{m}", name=f"qt{m}") for m in range(HP)]
            kt8 = [res.tile([P, 2, N], qk_dt, tag=f"kt{m}", name=f"kt{m}") for m in range(HP)]
            v8p = [res.tile([P, 2, H], qk_dt, tag=f"v8{m}", name=f"v8{m}") for m in range(NP2)] if fp8 else []
            vt = [res.tile([P, H], BF16, tag=f"v{n}", name=f"v{n}") for n in range(4 if fp8 else NT)]
            # bf16 head copies of q^T/k^T (queries/keys 0..127, h-major free)
            qhead = const.tile([P, H], BF16, tag="qhead")
            khead = const.tile([P, H], BF16, tag="khead")

            # ---------------- Phase 1: projections (K, Q, then V) ----------------
            with ExitStack() as p1:
                xt_pool = p1.enter_context(tc.tile_pool(name="xt", bufs=1))
                w_pool = p1.enter_context(tc.tile_pool(name="w", bufs=1))
                stg = p1.enter_context(tc.tile_pool(name="stg", bufs=2))
                ps1 = p1.enter_context(tc.tile_pool(name="ps1", bufs=4, space="PSUM"))

                bv_sb = stg.tile([P, H], F32, tag="bv", bufs=1)

                xt8 = [xt_pool.tile([P, 2, N], qk_dt, tag=f"x8{m}", name=f"x8{m}") for m in range(DP)]
                xtb = [xt_pool.tile([P, 512], BF16, tag=f"xb{d}", name=f"xb{d}") for d in range(DT)]
                wk = [w_pool.tile([P, 2, H], qk_dt, tag=f"wk{m}", name=f"wk{m}") for m in range(DP)]
                wq = [w_pool.tile([P, 2, H], qk_dt, tag=f"wq{m}", name=f"wq{m}") for m in range(DP)]
                wv8 = [w_pool.tile([P, 2, H], qk_dt, tag=f"wv8{m}", name=f"wv8{m}") for m in range(DP)]
                wv = [w_pool.tile([P, H], BF16, tag=f"wv{d}", name=f"wv{d}") for d in range(DT)]
                wqb = [w_pool.tile([P, H], BF16, tag=f"wqb{d}", name=f"wqb{d}") for d in range(DT)]
                wkb = [w_pool.tile([P, H], BF16, tag=f"wkb{d}", name=f"wkb{d}") for d in range(DT)]

                # DMA issue split across sync+gpsimd so descriptor issuance
                # parallelizes (ACT/DVE stay free for psum evictions). The
                # very first K-proj group only reads wk[*] h0-columns +
                # xt8[*] chunk 0, so those land first, split for queue
                # concurrency.
                for m in range(DP):
                    (nc.sync, nc.gpsimd, nc.scalar, nc.sync)[m].dma_start(
                        wk[m][:, :, 0:P], Wk8.ap()[m, :, :, 0:P]
                    )
                    eng = (nc.gpsimd, nc.scalar, nc.sync, nc.gpsimd)[m]
                    eng.dma_start(
                        xt8[m][0:64, :, 0:512], xT8.ap()[m, 0:64, :, 0:512]
                    )
                    (nc.scalar if m < 2 else eng).dma_start(
                        xt8[m][64:128, :, 0:512], xT8.ap()[m, 64:128, :, 0:512]
                    )
                for m in range(DP):
                    (nc.sync if m % 2 == 0 else nc.gpsimd).dma_start(
                        wk[m][:, :, P:H], Wk8.ap()[m, :, :, P:H]
                    )
                nc.gpsimd.dma_start(bk_sb[:], bkT.ap()[:, :])
                nc.gpsimd.dma_start(bq_sb[:], bqT.ap()[:, :])
                # Remaining x^T chunks, chunk-major so chunk c arrives before
                # the nch=c projection groups need it.
                for c in range(1, 4):
                    for m in range(DP):
                        eng = nc.sync if m % 2 == 0 else nc.gpsimd
                        eng.dma_start(
                            xt8[m][:, :, c * 512:(c + 1) * 512],
                            xT8.ap()[m, :, :, c * 512:(c + 1) * 512],
                        )
                for m in range(DP):
                    nc.sync.dma_start(wq[m][:], Wq8.ap()[m, :, :, :])
                for d in range(DT):
                    (nc.sync if d % 2 else nc.gpsimd).dma_start(
                        xtb[d][:], xTb.ap()[d, :, :]
                    )
                for d in range(DT):
                    (nc.sync if d % 2 else nc.gpsimd).dma_start(
                        wv[d][:], Wvb.ap()[d, :, :]
                    )
                for m in range(DP):
                    nc.gpsimd.dma_start(wv8[m][:], Wv8.ap()[m, :, :, :])
                nc.sync.dma_start(bv_sb[:], bvB.ap()[:, :])
                for d in range(DT):
                    (nc.sync if d % 2 else nc.gpsimd).dma_start(
                        wqb[d][:], Wqb.ap()[d, :, :]
                    )
                    (nc.gpsimd if d % 2 else nc.sync).dma_start(
                        wkb[d][:], Wkb.ap()[d, :, :]
                    )

                def proj_T(w, dst, bias):
                    """dst[h//2][:, h%2, :] = (W^T x^T + b) for all h, fp8/bf16.

                    nch-outer so the first groups only need x^T chunk 0."""
                    for nch in range(4):
                        for h in range(HT):
                            ps = ps1.tile([P, 512], F32, tag="ps")
                            if fp8:
                                for m in range(DP):
                                    nc.tensor.matmul(
                                        ps[:],
                                        w[m][:, :, h * P:(h + 1) * P],
                                        xt8[m][:, :, nch * 512:(nch + 1) * 512],
                                        start=(m == 0),
                                        stop=(m == DP - 1),
                                        perf_mode=DR,
                                    )
                            else:
                                for m in range(DP):
                                    for i in range(2):
                                        nc.tensor.matmul(
                                            ps[:],
                                            w[m][:, i, h * P:(h + 1) * P],
                                            xt8[m][:, i, nch * 512:(nch + 1) * 512],
                                            start=(m == 0 and i == 0),
                                            stop=(m == DP - 1 and i == 1),
                                        )
                            dstap = dst[h // 2][:, h % 2, nch * 512:(nch + 1) * 512]
                            if (h + nch) % 2 == 0:
                                nc.scalar.activation(
                                    dstap, ps[:], Identity, bias=bias[:, h:h + 1]
                                )
                            else:
                                nc.vector.tensor_scalar_add(dstap, ps[:], bias[:, h:h + 1])

                proj_T(wk, kt8, bk_sb)
                proj_T(wq, qt8, bq_sb)

                # --- V = x @ Wv (+ bv) ---
                # n-tiles 0..3 in bf16 (evicted to both bf16 vt and fp8 v8p);
                # n-tiles 4..15 in fp8-DR straight to v8p (their queries only
                # see them through the fp8 AV path).
                for n in range(NT):
                    for hch in range(2):
                        ps = ps1.tile([P, 512], F32, tag="ps")
                        hs = slice(hch * 512, (hch + 1) * 512)
                        if fp8 and n >= 4:
                            for m in range(DP):
                                nc.tensor.matmul(
                                    ps[:],
                                    xt8[m][:, :, n * P:(n + 1) * P],
                                    wv8[m][:, :, hs],
                                    start=(m == 0),
                                    stop=(m == DP - 1),
                                    perf_mode=DR,
                                )
                            nc.vector.tensor_add(
                                v8p[n // 2][:, n % 2, hs], ps[:], bv_sb[:, hs]
                            )
                        else:
                            for d in range(DT):
                                lhsT = (
                                    xtb[d][:, n * P:(n + 1) * P]
                                    if n < 4
                                    else xt8[d // 2][:, d % 2, n * P:(n + 1) * P]
                                )
                                nc.tensor.matmul(
                                    ps[:],
                                    lhsT,
                                    wv[d][:, hs],
                                    start=(d == 0),
                                    stop=(d == DT - 1),
                                )
                            nc.vector.tensor_add(vt[n][:, hs], ps[:], bv_sb[:, hs])
                            if fp8:
                                nc.vector.tensor_add(
                                    v8p[n // 2][:, n % 2, hs], ps[:], bv_sb[:, hs]
                                )

                # --- bf16 head projections: q^T/k^T columns 0..127 ---
                for src, dst, bias in ((wkb, khead, bk_sb), (wqb, qhead, bq_sb)):
                    for h in range(HT):
                        ps = ps1.tile([P, 512], F32, tag="ps")
                        for d in range(DT):
                            nc.tensor.matmul(
                                ps[:, 0:P],
                                src[d][:, h * P:(h + 1) * P],
                                xtb[d][:, 0:P],
                                start=(d == 0),
                                stop=(d == DT - 1),
                            )
                        if h % 2 == 0:
                            nc.scalar.activation(
                                dst[:, h * P:(h + 1) * P], ps[:, 0:P], Identity,
                                bias=bias[:, h:h + 1],
                            )
                        else:
                            nc.vector.tensor_scalar_add(
                                dst[:, h * P:(h + 1) * P], ps[:, 0:P], bias[:, h:h + 1]
                            )

            # ---------------- Phase 2: attention ----------------
            with ExitStack() as p2:
                pt_pool = p2.enter_context(tc.tile_pool(name="pt", bufs=1))
                out_pool = p2.enter_context(tc.tile_pool(name="op", bufs=2))
                sm_pool = p2.enter_context(tc.tile_pool(name="sm", bufs=4))
                ps_av = p2.enter_context(tc.tile_pool(name="psav", bufs=4, space="PSUM"))

                # bf16 head score tile: S^T[key 0:128, query 0:128]
                ps = ps_s.tile([P, 512], F32, tag="ps")
                for h in range(HT):
                    nc.tensor.matmul(
                        ps[:, 0:P],
                        khead[:, h * P:(h + 1) * P],
                        qhead[:, h * P:(h + 1) * P],
                        start=(h == 0),
                        stop=(h == HT - 1),
                    )
                pt_head = pt_pool.tile([P, P], BF16, tag="pth")
                nc.scalar.activation(pt_head[:], ps[:, 0:P], Exp, scale=float(SCALE))
                nc.gpsimd.affine_select(
                    out=pt_head[:],
                    in_=pt_head[:],
                    compare_op=mybir.AluOpType.is_ge,
                    fill=0.0,
                    base=0,
                    channel_multiplier=-1,
                    pattern=[[1, P]],
                )

                for t in range(IT):
                    i0 = 512 * t
                    jmax = 4 * t + 3
                    # t=0 probs/V stay bf16; t>=1 go fp8 (paired layout)
                    t8 = fp8 and t > 0

                    # scores^T [key j, query i] + exp + causal mask.
                    # Diagonal j-tiles only compute the causally-live column
                    # range [c, 512); columns below c are never read (the AV
                    # lhsT slice for i-sub s starts at 128*s >= c).
                    ptb, ptp = [], []
                    for j in range(jmax + 1):
                        c = max(0, j * P - i0)
                        w_ = 512 - c
                        ps = ps_s.tile([P, 512], F32, tag="ps")
                        if fp8:
                            for m in range(HP):
                                nc.tensor.matmul(
                                    ps[:, 0:w_],
                                    kt8[m][:, :, j * P:(j + 1) * P],
                                    qt8[m][:, :, i0 + c:i0 + 512],
                                    start=(m == 0),
                                    stop=(m == HP - 1),
                                    perf_mode=DR,
                                )
                        else:
                            for m in range(HP):
                                for i in range(2):
                                    nc.tensor.matmul(
                                        ps[:, 0:w_],
                                        kt8[m][:, i, j * P:(j + 1) * P],
                                        qt8[m][:, i, i0 + c:i0 + 512],
                                        start=(m == 0 and i == 0),
                                        stop=(m == HP - 1 and i == 1),
                                    )
                        if t8:
                            if j % 2 == 0:
                                pp = pt_pool.tile(
                                    [P, 2, 512], FP8, tag=f"ptp{j // 2}",
                                    name=f"ptp{j // 2}",
                                )
                                ptp.append(pp)
                            dst = ptp[j // 2][:, j % 2, c:512]
                        else:
                            p = pt_pool.tile(
                                [P, 512], BF16, tag=f"pt{j}", name=f"pt{j}"
                            )
                            ptb.append(p)
                            dst = p[:, c:512]
                        nc.scalar.activation(
                            dst, ps[:, 0:w_], Exp, scale=float(SCALE)
                        )
                        if c > 0 or j * P == i0:
                            # keep exp where key j*P+p <= query i0+c+f', else 0
                            nc.gpsimd.affine_select(
                                out=dst,
                                in_=dst,
                                compare_op=mybir.AluOpType.is_ge,
                                fill=0.0,
                                base=0,
                                channel_multiplier=-1,
                                pattern=[[1, w_]],
                            )

                    # attn @ V, row-sums, normalize on eviction
                    for s in range(4):
                        g = 4 * t + s
                        pav = [ps_av.tile([P, 512], F32, tag="pav", name="pav") for _ in range(2)]
                        prs = ps_rs.tile([P, 1], F32, tag="prs")
                        if t8:
                            # paired fp8 AV: DR over key pairs, plain fp8 for
                            # a trailing odd key tile. The very last output
                            # block runs hch-major so its first half's
                            # eviction + DMA overlap the second half's mms.
                            npair = (g + 1) // 2
                            lone = (g + 1) % 2
                            last = t == IT - 1 and s == 3
                            hch_outer = [[0, 1]] if not last else [[0], [1]]
                            for hchs in hch_outer:
                                for m in range(npair):
                                    lhsT = ptp[m][:, :, s * P:(s + 1) * P]
                                    for hch in hchs:
                                        nc.tensor.matmul(
                                            pav[hch][:],
                                            lhsT,
                                            v8p[m][:, :, hch * 512:(hch + 1) * 512],
                                            start=(m == 0),
                                            stop=(m == npair - 1 and not lone),
                                            perf_mode=DR,
                                        )
                                    if hchs[0] == 0:
                                        nc.tensor.matmul(
                                            prs[:],
                                            lhsT,
                                            ones8[:],
                                            start=(m == 0),
                                            stop=(m == npair - 1 and not lone),
                                            perf_mode=DR,
                                        )
                                if lone:
                                    lhsT = ptp[g // 2][:, g % 2, s * P:(s + 1) * P]
                                    for hch in hchs:
                                        nc.tensor.matmul(
                                            pav[hch][:],
                                            lhsT,
                                            v8p[g // 2][:, g % 2, hch * 512:(hch + 1) * 512],
                                            start=False,
                                            stop=True,
                                        )
                                    if hchs[0] == 0:
                                        nc.tensor.matmul(
                                            prs[:], lhsT, ones8[:, 0, :],
                                            start=False, stop=True,
                                        )
                        else:
                            for j in range(g + 1):
                                if t == 0 and s == 0:
                                    lhsT = pt_head[:]
                                else:
                                    lhsT = ptb[j][:, s * P:(s + 1) * P]
                                for hch in range(2):
                                    nc.tensor.matmul(
                                        pav[hch][:],
                                        lhsT,
                                        vt[j][:, hch * 512:(hch + 1) * 512],
                                        start=(j == 0),
                                        stop=(j == g),
                                    )
                                nc.tensor.matmul(
                                    prs[:],
                                    lhsT,
                                    ones[:],
                                    start=(j == 0),
                                    stop=(j == g),
                                )
                        recip = sm_pool.tile([P, 1], F32, tag="recip")
                        nc.vector.reciprocal(recip[:], prs[:])
                        ot = out_pool.tile([P, H], F32, tag="ot")
                        nc.scalar.activation(
                            ot[:, 0:512], pav[0][:], Copy, scale=recip[:]
                        )
                        nc.vector.tensor_scalar_mul(
                            ot[:, 512:1024], pav[1][:], recip[:]
                        )
                        r0 = i0 + s * P
                        if t < IT - 1:
                            # column halves issued as each eviction lands
                            for hch in range(2):
                                (nc.sync if hch == 0 else nc.gpsimd).dma_start(
                                    out.ap()[r0:r0 + P, hch * 512:(hch + 1) * 512],
                                    ot[:, hch * 512:(hch + 1) * 512],
                                )
                        else:
                            # tail blocks: quarter split across 4 DMA queues,
                            # issued from two engines in parallel
                            for hch in range(2):
                                eng = nc.sync if hch == 0 else nc.gpsimd
                                for ph in range(2):
                                    eng.dma_start(
                                        out.ap()[
                                            r0 + ph * 64:r0 + (ph + 1) * 64,
                                            hch * 512:(hch + 1) * 512,
                                        ],
                                        ot[ph * 64:(ph + 1) * 64,
                                           hch * 512:(hch + 1) * 512],
                                    )

    nc.compile()
    return nc


def _get_program():
    name = os.environ.get("ATTN_MODE", "fp8")
    if name not in _CACHE:
        _CACHE[name] = build_program(name)
    return _CACHE[name]


def kernel(x, Wq, bq, Wk, bk, Wv, bv):
    global LAST_RESULT
    x = np.asarray(x, dtype=np.float32)
    Wq = np.asarray(Wq, dtype=np.float32)
    Wk = np.asarray(Wk, dtype=np.float32)
    Wv = np.asarray(Wv, dtype=np.float32)
    bq = np.asarray(bq, dtype=np.float32)
    bk = np.asarray(bk, dtype=np.float32)
    bv = np.asarray(bv, dtype=np.float32)

    mode = os.environ.get("ATTN_MODE", "fp8")
    qk_np = ml_dtypes.float8_e4m3 if mode == "fp8" else ml_dtypes.bfloat16
    nc = _get_program()

    def pair_pack(mT, dtype):
        # [D, F] (d-major rows) -> [DP, 128, 2, F] DoubleRow pairs
        return np.ascontiguousarray(
            mT.reshape(DP, 2, P, -1).transpose(0, 2, 1, 3)
        ).astype(dtype)

    def dtile(mT):
        # [D, F] -> [DT, 128, F] bf16
        return np.ascontiguousarray(mT.reshape(DT, P, -1)).astype(ml_dtypes.bfloat16)

    Wq_p = pair_pack(Wq, qk_np)
    Wk_p = pair_pack(Wk, qk_np)
    Wv_p = pair_pack(Wv, qk_np)
    Wv_b = dtile(Wv)
    Wq_b = dtile(Wq)
    Wk_b = dtile(Wk)

    bqT = np.ascontiguousarray(bq.reshape(HT, P).T)
    bkT = np.ascontiguousarray(bk.reshape(HT, P).T)
    bvB = np.ascontiguousarray(np.broadcast_to(bv, (P, H)))

    in_maps = []
    for b in range(B):
        xT = np.ascontiguousarray(x[b].T)  # [D, N]
        in_maps.append(
            {
                "xT8": pair_pack(xT, qk_np),
                "Wq8": Wq_p,
                "Wk8": Wk_p,
                "Wv8": Wv_p,
                "xTb": dtile(xT[:, 0:512]),
                "Wvb": Wv_b,
                "Wqb": Wq_b,
                "Wkb": Wk_b,
                "bqT": bqT,
                "bkT": bkT,
                "bvB": bvB,
            }
        )

    res = run_bass_kernel_spmd(nc, in_maps, core_ids=list(range(B)))
    LAST_RESULT = res
    return np.stack([res.results[b]["out"] for b in range(B)], axis=0)


# revision 21
# speedup vs baseline: 1.0091x; 1.0091x over previous
"""Causal single-head attention (B=8, N=2048, D=H=1024, fp32) on 8 TRN2 cores.

Data-parallel: one batch element per NeuronCore. Fully SBUF-resident — no
DRAM spills. fp8-e4m3 with MatmulPerfMode.DoubleRow (two 128-deep contraction
slabs per matmul = 2x PE throughput) is used wherever max-abs accuracy
permits, tiered by how concentrated the causal attention can be:

  - queries 0..127   : bf16 everything (private head path) — they attend over
                       so few keys that fp8 score noise doesn't average out.
  - queries 128..511 : fp8-DR q/k projections + scores; bf16 probs/V.
  - queries 512..2047: fp8-DR q/k projections + scores, fp8 probs and fp8 V
                       (V rows >=512 also projected in fp8-DR).

Scores are computed transposed (S^T = K @ Q^T, [key, query]) so softmax
reduces over the partition (key) axis via 1-wide ones-matmuls, and the
normalization is folded into the output eviction as a per-partition scale.

ATTN_MODE=bf16 switches everything to bf16 (no fp8) as a fallback.
"""

import os
import sys
from contextlib import ExitStack

import numpy as np
import ml_dtypes

try:
    import concourse.bacc as bacc
except ImportError:  # pragma: no cover
    sys.path.insert(0, "/opt/trn_rl_repo")
    import concourse.bacc as bacc

import concourse.mybir as mybir
from concourse.tile import TileContext
from concourse.bass_utils import run_bass_kernel_spmd

# bass_utils imports antenv.axon_hooks when BASS_TRACE is set; provide a stub
# so tracing degrades gracefully instead of crashing if the module is absent.
try:
    import antenv.axon_hooks  # noqa: F401
except ImportError:  # pragma: no cover
    import types

    _m = types.ModuleType("antenv.axon_hooks")
    _m._hook = None
    _m.set_axon_ntff_profile_hook = lambda h: setattr(_m, "_hook", h)
    _m.get_axon_ntff_profile_hook = lambda: _m._hook
    sys.modules["antenv.axon_hooks"] = _m

B, N, D, H = 8, 2048, 1024, 1024
P = 128
DT = D // P          # 8 contraction tiles for the projections
DP = DT // 2         # 4 DoubleRow d-pairs
HT = H // P          # 8 h-tiles
HP = HT // 2         # 4 DoubleRow h-pairs
NT = N // P          # 16 sequence tiles of 128
NP2 = NT // 2        # 8 key-tile pairs
IT = N // 512        # 4 query tiles of 512
SCALE = 1.0 / np.sqrt(float(H))

F32 = mybir.dt.float32
BF16 = mybir.dt.bfloat16
FP8 = mybir.dt.float8e4

LAST_RESULT = None  # BassKernelResults of the most recent kernel() call
_CACHE = {}


def build_program(mode: str):
    fp8 = mode == "fp8"
    qk_dt = FP8 if fp8 else BF16
    DR = mybir.MatmulPerfMode.DoubleRow if fp8 else None

    nc = bacc.Bacc("TRN2", target_bir_lowering=False, debug=False)

    # Host packs x^T and Wq/Wk/Wv into DoubleRow d-pair layout:
    #   xT8[m, p, i, n] = x^T[(2m+i)*128 + p, n],  W8[m, p, i, h] likewise.
    xT8 = nc.dram_tensor("xT8", [DP, P, 2, N], qk_dt, kind="ExternalInput")
    Wq8 = nc.dram_tensor("Wq8", [DP, P, 2, H], qk_dt, kind="ExternalInput")
    Wk8 = nc.dram_tensor("Wk8", [DP, P, 2, H], qk_dt, kind="ExternalInput")
    Wv8 = nc.dram_tensor("Wv8", [DP, P, 2, H], qk_dt, kind="ExternalInput")
    xTb = nc.dram_tensor("xTb", [DT, P, 512], BF16, kind="ExternalInput")
    Wvb = nc.dram_tensor("Wvb", [DT, P, H], BF16, kind="ExternalInput")
    Wqb = nc.dram_tensor("Wqb", [DT, P, H], BF16, kind="ExternalInput")
    Wkb = nc.dram_tensor("Wkb", [DT, P, H], BF16, kind="ExternalInput")
    bqT = nc.dram_tensor("bqT", [P, HT], F32, kind="ExternalInput")
    bkT = nc.dram_tensor("bkT", [P, HT], F32, kind="ExternalInput")
    bvB = nc.dram_tensor("bvB", [P, H], F32, kind="ExternalInput")
    out = nc.dram_tensor("out", [N, H], F32, kind="ExternalOutput")

    Exp = mybir.ActivationFunctionType.Exp
    Identity = mybir.ActivationFunctionType.Identity
    Copy = mybir.ActivationFunctionType.Copy

    with TileContext(nc) as tc:
        with ExitStack() as top:
            const = top.enter_context(tc.tile_pool(name="const", bufs=1))
            res = top.enter_context(tc.tile_pool(name="res", bufs=1))
            ps_s = top.enter_context(tc.tile_pool(name="pss", bufs=3, space="PSUM"))
            ps_rs = top.enter_context(tc.tile_pool(name="psrs", bufs=1, space="PSUM"))

            ones = const.tile([P, 1], BF16, tag="ones")
            nc.vector.memset(ones[:], 1.0)
            ones8 = const.tile([P, 2, 1], qk_dt, tag="ones8")
            nc.vector.memset(ones8[:], 1.0)
            bq_sb = const.tile([P, HT], F32, tag="bq")
            bk_sb = const.tile([P, HT], F32, tag="bk")

            # Resident activations. qt8/kt8 hold the DoubleRow h-pair layout
            # ([:, i, :] = h-tile 2m+i); v8p the key-pair layout ([:, i, :] =
            # v rows (2m+i)*128..); vt holds bf16 v rows 0..511 for t=0.
            qt8 = [res.tile([P, 2, N], qk_dt, tag=f"qt{m}", name=f"qt{m}") for m in range(HP)]
            kt8 = [res.tile([P, 2, N], qk_dt, tag=f"kt{m}", name=f"kt{m}") for m in range(HP)]
            v8p = [res.tile([P, 2, H], qk_dt, tag=f"v8{m}", name=f"v8{m}") for m in range(NP2)] if fp8 else []
            vt = [res.tile([P, H], BF16, tag=f"v{n}", name=f"v{n}") for n in range(4 if fp8 else NT)]
            # bf16 head copies of q^T/k^T (queries/keys 0..127, h-major free)
            qhead = const.tile([P, H], BF16, tag="qhead")
            khead = const.tile([P, H], BF16, tag="khead")

            # ---------------- Phase 1: projections (K, Q, then V) ----------------
            with ExitStack() as p1:
                xt_pool = p1.enter_context(tc.tile_pool(name="xt", bufs=1))
                w_pool = p1.enter_context(tc.tile_pool(name="w", bufs=1))
                stg = p1.enter_context(tc.tile_pool(name="stg", bufs=2))
                ps1 = p1.enter_context(tc.tile_pool(name="ps1", bufs=4, space="PSUM"))

                bv_sb = stg.tile([P, H], F32, tag="bv", bufs=1)

                xt8 = [xt_pool.tile([P, 2, N], qk_dt, tag=f"x8{m}", name=f"x8{m}") for m in range(DP)]
                xtb = [xt_pool.tile([P, 512], BF16, tag=f"xb{d}", name=f"xb{d}") for d in range(DT)]
                wk = [w_pool.tile([P, 2, H], qk_dt, tag=f"wk{m}", name=f"wk{m}") for m in range(DP)]
                wq = [w_pool.tile([P, 2, H], qk_dt, tag=f"wq{m}", name=f"wq{m}") for m in range(DP)]
                wv8 = [w_pool.tile([P, 2, H], qk_dt, tag=f"wv8{m}", name=f"wv8{m}") for m in range(DP)]
                wv = [w_pool.tile([P, H], BF16, tag=f"wv{d}", name=f"wv{d}") for d in range(DT)]
                wqb = [w_pool.tile([P, H], BF16, tag=f"wqb{d}", name=f"wqb{d}") for d in range(DT)]
                wkb = [w_pool.tile([P, H], BF16, tag=f"wkb{d}", name=f"wkb{d}") for d in range(DT)]

                # DMA issue split across sync+gpsimd so descriptor issuance
                # parallelizes (ACT/DVE stay free for psum evictions). The
                # very first K-proj group only reads wk[*] h0-columns +
                # xt8[*] chunk 0, so those land first, split for queue
                # concurrency.
                for m in range(DP):
                    (nc.sync, nc.gpsimd, nc.scalar, nc.sync)[m].dma_start(
                        wk[m][:, :, 0:P], Wk8.ap()[m, :, :, 0:P]
                    )
                    eng = (nc.gpsimd, nc.scalar, nc.sync, nc.gpsimd)[m]
                    eng.dma_start(
                        xt8[m][0:64, :, 0:512], xT8.ap()[m, 0:64, :, 0:512]
                    )
                    (nc.scalar if m < 2 else eng).dma_start(
                        xt8[m][64:128, :, 0:512], xT8.ap()[m, 64:128, :, 0:512]
                    )
                for m in range(DP):
                    (nc.sync if m % 2 == 0 else nc.gpsimd).dma_start(
                        wk[m][:, :, P:H], Wk8.ap()[m, :, :, P:H]
                    )
                nc.gpsimd.dma_start(bk_sb[:], bkT.ap()[:, :])
                nc.gpsimd.dma_start(bq_sb[:], bqT.ap()[:, :])
                # Remaining x^T chunks, chunk-major so chunk c arrives before
                # the nch=c projection groups need it.
                for c in range(1, 4):
                    for m in range(DP):
                        eng = nc.sync if m % 2 == 0 else nc.gpsimd
                        eng.dma_start(
                            xt8[m][:, :, c * 512:(c + 1) * 512],
                            xT8.ap()[m, :, :, c * 512:(c + 1) * 512],
                        )
                for m in range(DP):
                    nc.sync.dma_start(wq[m][:], Wq8.ap()[m, :, :, :])
                for d in range(DT):
                    (nc.sync if d % 2 else nc.gpsimd).dma_start(
                        xtb[d][:], xTb.ap()[d, :, :]
                    )
                for d in range(DT):
                    (nc.sync if d % 2 else nc.gpsimd).dma_start(
                        wv[d][:], Wvb.ap()[d, :, :]
                    )
                for m in range(DP):
                    nc.gpsimd.dma_start(wv8[m][:], Wv8.ap()[m, :, :, :])
                nc.sync.dma_start(bv_sb[:], bvB.ap()[:, :])
                for d in range(DT):
                    (nc.sync if d % 2 else nc.gpsimd).dma_start(
                        wqb[d][:], Wqb.ap()[d, :, :]
                    )
                    (nc.gpsimd if d % 2 else nc.sync).dma_start(
                        wkb[d][:], Wkb.ap()[d, :, :]
                    )

                def proj_T(w, dst, bias):
                    """dst[h//2][:, h%2, :] = (W^T x^T + b) for all h, fp8/bf16.

                    nch-outer so the first groups only need x^T chunk 0."""
                    for nch in range(4):
                        for h in range(HT):
                            ps = ps1.tile([P, 512], F32, tag="ps")
                            if fp8:
                                for m in range(DP):
                                    nc.tensor.matmul(
                                        ps[:],
                                        w[m][:, :, h * P:(h + 1) * P],
                                        xt8[m][:, :, nch * 512:(nch + 1) * 512],
                                        start=(m == 0),
                                        stop=(m == DP - 1),
                                        perf_mode=DR,
                                    )
                            else:
                                for m in range(DP):
                                    for i in range(2):
                                        nc.tensor.matmul(
                                            ps[:],
                                            w[m][:, i, h * P:(h + 1) * P],
                                            xt8[m][:, i, nch * 512:(nch + 1) * 512],
                                            start=(m == 0 and i == 0),
                                            stop=(m == DP - 1 and i == 1),
                                        )
                            dstap = dst[h // 2][:, h % 2, nch * 512:(nch + 1) * 512]
                            if (h + nch) % 2 == 0:
                                nc.scalar.activation(
                                    dstap, ps[:], Identity, bias=bias[:, h:h + 1]
                                )
                            else:
                                nc.vector.tensor_scalar_add(dstap, ps[:], bias[:, h:h + 1])

                proj_T(wk, kt8, bk_sb)
                proj_T(wq, qt8, bq_sb)

                # --- V = x @ Wv (+ bv) ---
                # n-tiles 0..3 in bf16 (evicted to both bf16 vt and fp8 v8p);
                # n-tiles 4..15 in fp8-DR straight to v8p (their queries only
                # see them through the fp8 AV path).
                for n in range(NT):
                    for hch in range(2):
                        ps = ps1.tile([P, 512], F32, tag="ps")
                        hs = slice(hch * 512, (hch + 1) * 512)
                        if fp8 and n >= 4:
                            for m in range(DP):
                                nc.tensor.matmul(
                                    ps[:],
                                    xt8[m][:, :, n * P:(n + 1) * P],
                                    wv8[m][:, :, hs],
                                    start=(m == 0),
                                    stop=(m == DP - 1),
                                    perf_mode=DR,
                                )
                            nc.vector.tensor_add(
                                v8p[n // 2][:, n % 2, hs], ps[:], bv_sb[:, hs]
                            )
                        else:
                            for d in range(DT):
                                lhsT = (
                                    xtb[d][:, n * P:(n + 1) * P]
                                    if n < 4
                                    else xt8[d // 2][:, d % 2, n * P:(n + 1) * P]
                                )
                                nc.tensor.matmul(
                                    ps[:],
                                    lhsT,
                                    wv[d][:, hs],
                                    start=(d == 0),
                                    stop=(d == DT - 1),
                                )
                            nc.vector.tensor_add(vt[n][:, hs], ps[:], bv_sb[:, hs])
                            if fp8:
                                nc.vector.tensor_add(
                                    v8p[n // 2][:, n % 2, hs], ps[:], bv_sb[:, hs]
                                )

                # --- bf16 head projections: q^T/k^T columns 0..127 ---
                for src, dst, bias in ((wkb, khead, bk_sb), (wqb, qhead, bq_sb)):
                    for h in range(HT):
                        ps = ps1.tile([P, 512], F32, tag="ps")
                        for d in range(DT):
                            nc.tensor.matmul(
                                ps[:, 0:P],
                                src[d][:, h * P:(h + 1) * P],
                                xtb[d][:, 0:P],
                                start=(d == 0),
                                stop=(d == DT - 1),
                            )
                        if h % 2 == 0:
                            nc.scalar.activation(
                                dst[:, h * P:(h + 1) * P], ps[:, 0:P], Identity,
                                bias=bias[:, h:h + 1],
                            )
                        else:
                            nc.vector.tensor_scalar_add(
                                dst[:, h * P:(h + 1) * P], ps[:, 0:P], bias[:, h:h + 1]
                            )

            # ---------------- Phase 2: attention ----------------
            with ExitStack() as p2:
                pt_pool = p2.enter_context(tc.tile_pool(name="pt", bufs=1))
                out_pool = p2.enter_context(tc.tile_pool(name="op", bufs=2))
                sm_pool = p2.enter_context(tc.tile_pool(name="sm", bufs=4))
                ps_av = p2.enter_context(tc.tile_pool(name="psav", bufs=4, space="PSUM"))

                # bf16 head score tile: S^T[key 0:128, query 0:128]
                ps = ps_s.tile([P, 512], F32, tag="ps")
                for h in range(HT):
                    nc.tensor.matmul(
                        ps[:, 0:P],
                        khead[:, h * P:(h + 1) * P],
                        qhead[:, h * P:(h + 1) * P],
                        start=(h == 0),
                        stop=(h == HT - 1),
                    )
                pt_head = pt_pool.tile([P, P], BF16, tag="pth")
                nc.scalar.activation(pt_head[:], ps[:, 0:P], Exp, scale=float(SCALE))
                nc.gpsimd.affine_select(
                    out=pt_head[:],
                    in_=pt_head[:],
                    compare_op=mybir.AluOpType.is_ge,
                    fill=0.0,
                    base=0,
                    channel_multiplier=-1,
                    pattern=[[1, P]],
                )

                for t in range(IT):
                    i0 = 512 * t
                    jmax = 4 * t + 3
                    # t=0 probs/V stay bf16; t>=1 go fp8 (paired layout)
                    t8 = fp8 and t > 0

                    # scores^T [key j, query i] + exp + causal mask.
                    # Diagonal j-tiles only compute the causally-live column
                    # range [c, 512); columns below c are never read (the AV
                    # lhsT slice for i-sub s starts at 128*s >= c).
                    ptb, ptp = [], []
                    for j in range(jmax + 1):
                        c = max(0, j * P - i0)
                        w_ = 512 - c
                        ps = ps_s.tile([P, 512], F32, tag="ps")
                        if fp8:
                            for m in range(HP):
                                nc.tensor.matmul(
                                    ps[:, 0:w_],
                                    kt8[m][:, :, j * P:(j + 1) * P],
                                    qt8[m][:, :, i0 + c:i0 + 512],
                                    start=(m == 0),
                                    stop=(m == HP - 1),
                                    perf_mode=DR,
                                )
                        else:
                            for m in range(HP):
                                for i in range(2):
                                    nc.tensor.matmul(
                                        ps[:, 0:w_],
                                        kt8[m][:, i, j * P:(j + 1) * P],
                                        qt8[m][:, i, i0 + c:i0 + 512],
                                        start=(m == 0 and i == 0),
                                        stop=(m == HP - 1 and i == 1),
                                    )
                        if t8:
                            if j % 2 == 0:
                                pp = pt_pool.tile(
                                    [P, 2, 512], FP8, tag=f"ptp{j // 2}",
                                    name=f"ptp{j // 2}",
                                )
                                ptp.append(pp)
                            dst = ptp[j // 2][:, j % 2, c:512]
                        else:
                            p = pt_pool.tile(
                                [P, 512], BF16, tag=f"pt{j}", name=f"pt{j}"
                            )
                            ptb.append(p)
                            dst = p[:, c:512]
                        nc.scalar.activation(
                            dst, ps[:, 0:w_], Exp, scale=float(SCALE)
                        )
                        if c > 0 or j * P == i0:
                            # keep exp where key j*P+p <= query i0+c+f', else 0
                            nc.gpsimd.affine_select(
                                out=dst,
                                in_=dst,
                                compare_op=mybir.AluOpType.is_ge,
                                fill=0.0,
                                base=0,
                                channel_multiplier=-1,
                                pattern=[[1, w_]],
                            )

                    # attn @ V, row-sums, normalize on eviction
                    for s in range(4):
                        g = 4 * t + s
                        pav = [ps_av.tile([P, 512], F32, tag="pav", name="pav") for _ in range(2)]
                        prs = ps_rs.tile([P, 1], F32, tag="prs")
                        if t8:
                            # paired fp8 AV: DR over key pairs, plain fp8 for
                            # a trailing odd key tile. The very last output
                            # block runs hch-major so its first half's
                            # eviction + DMA overlap the second half's mms.
                            npair = (g + 1) // 2
                            lone = (g + 1) % 2
                            last = t == IT - 1 and s == 3
                            hch_outer = [[0, 1]] if not last else [[0], [1]]
                            for hchs in hch_outer:
                                for m in range(npair):
                                    lhsT = ptp[m][:, :, s * P:(s + 1) * P]
                                    # order hch0, rowsum, hch1: the 1-wide
                                    # rowsum's LDWEIGHTS hides under hch0's
                                    # exec instead of stalling the next pair
                                    nc.tensor.matmul(
                                        pav[hchs[0]][:],
                                        lhsT,
                                        v8p[m][:, :, hchs[0] * 512:(hchs[0] + 1) * 512],
                                        start=(m == 0),
                                        stop=(m == npair - 1 and not lone),
                                        perf_mode=DR,
                                    )
                                    if hchs[0] == 0:
                                        nc.tensor.matmul(
                                            prs[:],
                                            lhsT,
                                            ones8[:],
                                            start=(m == 0),
                                            stop=(m == npair - 1 and not lone),
                                            perf_mode=DR,
                                        )
                                    for hch in hchs[1:]:
                                        nc.tensor.matmul(
                                            pav[hch][:],
                                            lhsT,
                                            v8p[m][:, :, hch * 512:(hch + 1) * 512],
                                            start=(m == 0),
                                            stop=(m == npair - 1 and not lone),
                                            perf_mode=DR,
                                        )
                                if lone:
                                    lhsT = ptp[g // 2][:, g % 2, s * P:(s + 1) * P]
                                    nc.tensor.matmul(
                                        pav[hchs[0]][:],
                                        lhsT,
                                        v8p[g // 2][:, g % 2,
                                            hchs[0] * 512:(hchs[0] + 1) * 512],
                                        start=False,
                                        stop=True,
                                    )
                                    if hchs[0] == 0:
                                        nc.tensor.matmul(
                                            prs[:], lhsT, ones8[:, 0, :],
                                            start=False, stop=True,
                                        )
                                    for hch in hchs[1:]:
                                        nc.tensor.matmul(
                                            pav[hch][:],
                                            lhsT,
                                            v8p[g // 2][:, g % 2,
                                                hch * 512:(hch + 1) * 512],
                                            start=False,
                                            stop=True,
                                        )
                        else:
                            for j in range(g + 1):
                                if t == 0 and s == 0:
                                    lhsT = pt_head[:]
                                else:
                                    lhsT = ptb[j][:, s * P:(s + 1) * P]
                                nc.tensor.matmul(
                                    pav[0][:],
                                    lhsT,
                                    vt[j][:, 0:512],
                                    start=(j == 0),
                                    stop=(j == g),
                                )
                                nc.tensor.matmul(
                                    prs[:],
                                    lhsT,
                                    ones[:],
                                    start=(j == 0),
                                    stop=(j == g),
                                )
                                nc.tensor.matmul(
                                    pav[1][:],
                                    lhsT,
                                    vt[j][:, 512:1024],
                                    start=(j == 0),
                                    stop=(j == g),
                                )
                        recip = sm_pool.tile([P, 1], F32, tag="recip")
                        nc.vector.reciprocal(recip[:], prs[:])
                        ot = out_pool.tile([P, H], F32, tag="ot")
                        nc.scalar.activation(
                            ot[:, 0:512], pav[0][:], Copy, scale=recip[:]
                        )
                        nc.vector.tensor_scalar_mul(
                            ot[:, 512:1024], pav[1][:], recip[:]
                        )
                        r0 = i0 + s * P
                        if t < IT - 1:
                            # column halves issued as each eviction lands
                            for hch in range(2):
                                (nc.sync if hch == 0 else nc.gpsimd).dma_start(
                                    out.ap()[r0:r0 + P, hch * 512:(hch + 1) * 512],
                                    ot[:, hch * 512:(hch + 1) * 512],
                                )
                        else:
                            # tail blocks: quarter split across 4 DMA queues,
                            # issued from three engines in parallel (scalar
                            # has just produced the first half itself)
                            engs = (nc.scalar, nc.sync, nc.gpsimd, nc.sync)
                            for hch in range(2):
                                for ph in range(2):
                                    engs[2 * hch + ph].dma_start(
                                        out.ap()[
                                            r0 + ph * 64:r0 + (ph + 1) * 64,
                                            hch * 512:(hch + 1) * 512,
                                        ],
                                        ot[ph * 64:(ph + 1) * 64,
                                           hch * 512:(hch + 1) * 512],
                                    )

    nc.compile()
    return nc


def _get_program():
    name = os.environ.get("ATTN_MODE", "fp8")
    if name not in _CACHE:
        _CACHE[name] = build_program(name)
    return _CACHE[name]


def kernel(x, Wq, bq, Wk, bk, Wv, bv):
    global LAST_RESULT
    x = np.asarray(x, dtype=np.float32)
    Wq = np.asarray(Wq, dtype=np.float32)
    Wk = np.asarray(Wk, dtype=np.float32)
    Wv = np.asarray(Wv, dtype=np.float32)
    bq = np.asarray(bq, dtype=np.float32)
    bk = np.asarray(bk, dtype=np.float32)
    bv = np.asarray(bv, dtype=np.float32)

    mode = os.environ.get("ATTN_MODE", "fp8")
    qk_np = ml_dtypes.float8_e4m3 if mode == "fp8" else ml_dtypes.bfloat16
    nc = _get_program()

    def pair_pack(mT, dtype):
        # [D, F] (d-major rows) -> [DP, 128, 2, F] DoubleRow pairs
        return np.ascontiguousarray(
            mT.reshape(DP, 2, P, -1).transpose(0, 2, 1, 3)
        ).astype(dtype)

    def dtile(mT):
        # [D, F] -> [DT, 128, F] bf16
        return np.ascontiguousarray(mT.reshape(DT, P, -1)).astype(ml_dtypes.bfloat16)

    Wq_p = pair_pack(Wq, qk_np)
    Wk_p = pair_pack(Wk, qk_np)
    Wv_p = pair_pack(Wv, qk_np)
    Wv_b = dtile(Wv)
    Wq_b = dtile(Wq)
    Wk_b = dtile(Wk)

    bqT = np.ascontiguousarray(bq.reshape(HT, P).T)
    bkT = np.ascontiguousarray(bk.reshape(HT, P).T)
    bvB = np.ascontiguousarray(np.broadcast_to(bv, (P, H)))

    in_maps = []
    for b in range(B):
        xT = np.ascontiguousarray(x[b].T)  # [D, N]
        in_maps.append(
            {
                "xT8": pair_pack(xT, qk_np),
                "Wq8": Wq_p,
                "Wk8": Wk_p,
                "Wv8": Wv_p,
                "xTb": dtile(xT[:, 0:512]),
                "Wvb": Wv_b,
                "Wqb": Wq_b,
                "Wkb": Wk_b,
                "bqT": bqT,
                "bkT": bkT,
                "bvB": bvB,
            }
        )

    res = run_bass_kernel_spmd(nc, in_maps, core_ids=list(range(B)))
    LAST_RESULT = res
    return np.stack([res.results[b]["out"] for b in range(B)], axis=0)
